# revision 25
# baseline (speedup 1.0000x reference)
"""CQAttention Trainium2 kernel (fp16 fast path).

Full inputs -> full output; internally data-parallel over batch B=32 across
8 NeuronCores (NB=4 batch items per core).

Math (per batch item, d=128, Lc=2048, Lq=256, all-ones masks):
  S[i,j] = (C@w_c)[i] + (Q@w_q)[j] + b + (C*w_m)[i] @ Q[j]
  E = exp(S); s1_j = sum_i E; s2_i = sum_j E
  C2Q = (E/s1) @ Q ; T = (E/s2)^T @ C ; Q2C = (E/s1) @ T
  out = concat([C, C2Q, C*C2Q, C*Q2C], -1)

Device decomposition (exp without max-subtraction is safe: |S| <~ 6):
  qm'[d,j] = w_m[d]*Q[j,d] + w_c[d]   (so qm'^T C^T = S_mm + r_i rides the MM)
  ht[j,i]  = exp(qm'^T@C^T + qb_j + b) = E^T   (ACT exp, bias per partition,
             accum -> s1)
  G[i,j]   = PE-transpose of ht (no second exp pass); s2 = DVE reduce of G
  T^T[d,j] = (C/s2)^T @ G ; Tw[j,d] = T/s1 via PE transpose + scale
  [C2Q|Q2C][i,:] = sum_j ht[j,i]*[Q/s1 | Tw][j,:]   (er/eq factors inside E)
  col2 = C*C2Q (gpsimd), col3 = C*Q2C (DVE, from PSUM)

All matmuls run in float16 (1 PE cycle/row at 2.4 GHz). I/O is fp16; host
converts. Tolerance is 2e-2 fro; fp16 end-to-end lands ~1e-3.
"""

import numpy as np

import concourse.bass as bass
import concourse.mybir as mybir
import concourse.tile as tile
import concourse.bacc as bacc
from concourse import masks as cmasks
from concourse.bass_utils import run_bass_kernel_spmd

F32 = mybir.dt.float32
F16 = mybir.dt.float16
AF = mybir.ActivationFunctionType
ALU = mybir.AluOpType
AX = mybir.AxisListType

N_CORES = 8
D = 128


def build_nc(NB=4, Lc=2048, Lq=256):
    NT = Lc // 128   # 16 i-tiles
    NJ = Lq // 128   # 2 j-tiles

    nc = bacc.Bacc()
    CT = nc.declare_dram_parameter("CT", [NB, 128, Lc], F16, isOutput=False)
    CN = nc.declare_dram_parameter("CN", [NB, 128, Lc], F16, isOutput=False)
    QT = nc.declare_dram_parameter("QT", [NB, 128, Lq], F16, isOutput=False)
    WC = nc.declare_dram_parameter("WC", [128, 1], F32, isOutput=False)
    WM = nc.declare_dram_parameter("WM", [128, 1], F32, isOutput=False)
    QB = nc.declare_dram_parameter("QB", [128, NB * 2], F32, isOutput=False)
    BR = nc.declare_dram_parameter("BR", [128, 1], F32, isOutput=False)
    OUT = nc.declare_dram_parameter("OUT", [NB, Lc, 384], F16, isOutput=True)

    with tile.TileContext(nc) as tc:
        import contextlib
        with contextlib.ExitStack() as ctx:
            const = ctx.enter_context(tc.tile_pool(name="const", bufs=1))
            pin = ctx.enter_context(tc.tile_pool(name="pin", bufs=4))
            mid = ctx.enter_context(tc.tile_pool(name="mid", bufs=2))
            psHT = ctx.enter_context(tc.tile_pool(name="psHT", bufs=2, space="PSUM"))
            psGT = ctx.enter_context(tc.tile_pool(name="psGT", bufs=3, space="PSUM"))
            psT = ctx.enter_context(tc.tile_pool(name="psT", bufs=1, space="PSUM"))
            psF = ctx.enter_context(tc.tile_pool(name="psF", bufs=2, space="PSUM"))

            # ---- constants ----
            wc_col = const.tile([128, 1], F32)
            nc.sync.dma_start(wc_col[:], WC[:])
            wm_col = const.tile([128, 1], F32)
            nc.sync.dma_start(wm_col[:], WM[:])
            b_rep = const.tile([128, 1], F32)
            nc.sync.dma_start(b_rep[:], BR[:])
            ones16 = const.tile([1, 128], F16)
            nc.vector.memset(ones16[:], 1.0)
            wrhs = const.tile([1, 512], F16)
            nc.vector.memset(wrhs[:], 1.0)
            ident = const.tile([128, 128], F16)
            cmasks.make_identity(nc, ident[:])
            qbb_all = const.tile([128, NB * 2], F32)
            nc.sync.dma_start(qbb_all[:], QB[:])

            # per-batch state handles
            st = [dict() for _ in range(NB)]

            def loads(bi):
                s = st[bi]
                qt = pin.tile([128, Lq], F16, tag="qt")
                nc.sync.dma_start(qt[:], QT[bi])
                ct = pin.tile([128, Lc], F16, tag="ct")
                nc.sync.dma_start(ct[:], CT[bi])
                cn = pin.tile([128, Lc], F16, tag="cn")
                nc.sync.dma_start(cn[:], CN[bi])
                s["qt"], s["ct"], s["cn"] = qt, ct, cn
                s["qbb"] = qbb_all[:, bi * 2:(bi + 1) * 2]

            def prep(bi):
                # qm' = qt*wm + wc ; qb cols ; qbb = qb + b
                s = st[bi]
                qm = mid.tile([128, Lq], F16, tag="qm")
                nc.gpsimd.tensor_scalar(qm[:], s["qt"][:], wm_col[:], wc_col[:],
                                        ALU.mult, ALU.add)
                ht = mid.tile([128, NJ * Lc], F16, tag="ht")
                s1p = mid.tile([128, NJ * 4], F32, tag="s1p")
                rs2 = mid.tile([128, NT], F16, tag="rs2")
                s["qm"], s["ht"], s["s1p"], s["rs2"] = qm, ht, s1p, rs2
                s["Cs"] = mid.tile([128, Lc], F16, tag="Cs", name="Cs")

            def ht_unit(bi, jj, g):
                # one 512-wide score chunk: MM + exp evac (+ s1 accum)
                s = st[bi]
                pg = psHT.tile([128, 512], F32, tag="ht")
                nc.tensor.matmul(pg[:], s["qm"][:, jj * 128:(jj + 1) * 128],
                                 s["ct"][:, g * 512:(g + 1) * 512],
                                 start=True, stop=True)
                nc.scalar.activation(
                    s["ht"][:, jj * Lc + g * 512: jj * Lc + (g + 1) * 512],
                    pg[:], AF.Exp, bias=s["qbb"][:, jj:jj + 1],
                    accum_out=s["s1p"][:, jj * 4 + g: jj * 4 + g + 1])

            def s1_fin(bi):
                s = st[bi]
                s1 = mid.tile([128, NJ], F32, tag="s1")
                nc.vector.tensor_reduce(
                    s1[:], s["s1p"][:].rearrange("p (j g) -> p j g", g=4),
                    AX.X, ALU.add)
                rs1 = mid.tile([128, NJ], F32, tag="rs1")
                nc.vector.reciprocal(rs1[:], s1[:])
                s["rs1"] = rs1

            def gt_unit(bi, c):
                # transpose 4 i-tiles (8 blocks) of ht into G via PE, then
                # per-tile DVE evac fused with s2 row-sum (TTR, 2x mode)
                s = st[bi]
                if c == 0:
                    s["G"] = mid.tile([128, NT * Lq], F16, tag="G", name="G")
                    s["s2p"] = mid.tile([128, NT], F16, tag="s2p", name="s2p")
                pg = psGT.tile([128, 1024], F16, tag="gt")
                for u in range(4):
                    t = c * 4 + u
                    for jj in range(NJ):
                        nc.tensor.matmul(
                            pg[:, u * 256 + jj * 128: u * 256 + (jj + 1) * 128],
                            s["ht"][:, jj * Lc + t * 128: jj * Lc + (t + 1) * 128],
                            ident[:], is_transpose=True)
                if c % 2 == 0:
                    nc.vector.tensor_copy(s["G"][:, c * 1024:(c + 1) * 1024],
                                          pg[:])
                else:
                    nc.scalar.activation(s["G"][:, c * 1024:(c + 1) * 1024],
                                         pg[:], AF.Copy)
                with nc.allow_low_precision("s2 ~ 4e2, fp16 out is 6e-4 rel"):
                    nc.vector.tensor_reduce(
                        s["s2p"][:, c * 4:(c + 1) * 4],
                        s["G"][:, c * 1024:(c + 1) * 1024]
                            .rearrange("p (t j) -> p t j", j=Lq),
                        AX.X, ALU.add)
                    nc.vector.reciprocal(s["rs2"][:, c * 4:(c + 1) * 4],
                                         s["s2p"][:, c * 4:(c + 1) * 4])

            def cs_unit(bi, c):
                # Cs = C / s2 for 4 i-tiles (gpsimd)
                s = st[bi]
                ts = slice(c * 4, (c + 1) * 4)
                nc.gpsimd.tensor_tensor(
                    s["Cs"][:].rearrange("p (t d) -> p t d", d=128)[:, ts, :],
                    s["cn"][:].rearrange("p (t d) -> p t d", d=128)[:, ts, :],
                    s["rs2"][:].rearrange("p t -> p t ()")[:, ts, :]
                        .broadcast_to((128, 4, 128)),
                    ALU.mult)

            def tt_unit(bi, k):
                # two T^T accumulation matmuls
                s = st[bi]
                if k == 0:
                    s["psT"] = psT.tile([128, Lq], F32, tag="tt", name="psTT")
                for t in (2 * k, 2 * k + 1):
                    nc.tensor.matmul(s["psT"][:], s["Cs"][:, t * 128:(t + 1) * 128],
                                     s["G"][:, t * Lq:(t + 1) * Lq],
                                     start=(t == 0), stop=(t == NT - 1),
                                     skip_group_check=True)

            def te_unit(bi):
                s = st[bi]
                Tt = mid.tile([128, Lq], F16, tag="Tt")
                nc.vector.tensor_copy(Tt[:], s["psT"][:])
                s["Tt"] = Tt

            def tr_unit(bi):
                # transposes: T^T -> Tw (scaled by 1/s1), qt -> Qs (scaled)
                s = st[bi]
                qtw = mid.tile([128, NJ, 256], F16, tag="qtw")
                ptrps = psGT.tile([128, 1024], F16, tag="gt", name="ptrps")
                ptr = ptrps[:, 0:256]
                for jj in range(NJ):
                    nc.tensor.matmul(ptr[:, jj * 128:(jj + 1) * 128],
                                     s["Tt"][:, jj * 128:(jj + 1) * 128],
                                     ident[:], is_transpose=True)
                pqs = ptrps[:, 512:768]
                for jj in range(NJ):
                    nc.tensor.matmul(pqs[:, jj * 128:(jj + 1) * 128],
                                     s["qt"][:, jj * 128:(jj + 1) * 128],
                                     ident[:], is_transpose=True)
                for jj in range(NJ):
                    nc.vector.tensor_scalar_mul(
                        qtw[:, jj, 128:256], ptr[:, jj * 128:(jj + 1) * 128],
                        s["rs1"][:, jj:jj + 1])
                    nc.vector.tensor_scalar_mul(
                        qtw[:, jj, 0:128], pqs[:, jj * 128:(jj + 1) * 128],
                        s["rs1"][:, jj:jj + 1])
                s["qtw"] = qtw
                s["big"] = mid.tile([128, NT, 384], F16, tag="big", name="big")

            def f_unit(bi, p):
                # fused C2Q/Q2C for tile pair (2p, 2p+1) + evac + products
                s = st[bi]
                pf = psF.tile([128, 512], F32, tag="f")
                for k in range(2):
                    t = 2 * p + k
                    for jj in range(NJ):
                        nc.tensor.matmul(
                            pf[:, k * 256:(k + 1) * 256],
                            s["ht"][:, jj * Lc + t * 128: jj * Lc + (t + 1) * 128],
                            s["qtw"][:, jj, :],
                            start=(jj == 0), stop=(jj == NJ - 1))
                pfv = pf[:].rearrange("p (k c) -> p k c", c=256)
                ts = slice(2 * p, 2 * p + 2)
                big, cn = s["big"], s["cn"]
                cnv = cn[:].rearrange("p (t d) -> p t d", d=128)
                if p >= 5:
                    nc.vector.tensor_copy(big[:, ts, 0:128], pfv[:, :, 0:128])
                else:
                    nc.scalar.activation(big[:, ts, 0:128], pfv[:, :, 0:128],
                                         AF.Copy)
                nc.vector.tensor_tensor(big[:, ts, 256:384], cnv[:, ts, :],
                                        pfv[:, :, 128:256], ALU.mult)
                if p == 7:
                    nc.vector.tensor_tensor(big[:, ts, 128:256], cnv[:, ts, :],
                                            big[:, ts, 0:128], ALU.mult)
                else:
                    nc.gpsimd.tensor_tensor(big[:, ts, 128:256], cnv[:, ts, :],
                                            big[:, ts, 0:128], ALU.mult)

            def store_unit(bi, q):
                s = st[bi]
                outv = OUT[bi].rearrange("(t p) c -> p t c", p=128)
                ts = slice(q * 4, (q + 1) * 4)
                nc.sync.dma_start(outv[:, ts, :], s["big"][:, ts, :])

            def stream1(bi):
                units = []
                for g in range(4):
                    for jj in range(NJ):
                        units.append(lambda b=bi, j=jj, g_=g: ht_unit(b, j, g_))
                    units.append(lambda b=bi, c_=g: gt_unit(b, c_))
                    if g > 0:
                        units.append(lambda b=bi, c_=g - 1: cs_unit(b, c_))
                units.append(lambda b=bi: s1_fin(b))
                units.append(lambda b=bi: cs_unit(b, 3))
                return units

            def stream2(bi):
                units = []
                for k in range(NT // 2):
                    units.append(lambda b=bi, k_=k: tt_unit(b, k_))
                units.append(lambda b=bi: te_unit(b))
                units.append(lambda b=bi: tr_unit(b))
                for p in range(NT // 2):
                    units.append(lambda b=bi, p_=p: f_unit(b, p_))
                    if p % 2 == 1:
                        units.append(lambda b=bi, q=p // 2: store_unit(b, q))
                return units

            # ---- prologue: loads for b0/b1, PE warm-up ----
            loads(0)
            for _k in range(14):
                pw = psHT.tile([128, 512], F32, tag="ht")
                nc.tensor.matmul(pw[:], ones16[:], wrhs[:], start=True, stop=True)
            loads(1)

            # ---- software-pipelined windows ----
            def interleave(a, b):
                out, ia, ib = [], 0, 0
                na, nb = len(a), len(b)
                while ia < na or ib < nb:
                    if ia < na:
                        out.append(a[ia]); ia += 1
                    if ib < nb:
                        out.append(b[ib]); ib += 1
                return out

            prep(0)
            prev = []
            for bi in range(NB):
                if bi + 2 <= NB - 1:
                    loads(bi + 2)
                if bi + 1 <= NB - 1:
                    prep(bi + 1)
                for u in interleave(stream1(bi), prev):
                    u()
                prev = stream2(bi)
            for u in prev:
                u()

    nc.finalize()
    return nc


_NC_CACHE = {}
LAST_RESULTS = None


def _get_nc(NB, Lc, Lq):
    key = (NB, Lc, Lq)
    if key not in _NC_CACHE:
        _NC_CACHE[key] = build_nc(NB, Lc, Lq)
    return _NC_CACHE[key]


def kernel(C, Q, w, b, c_mask, q_mask):
    C = np.ascontiguousarray(np.asarray(C), dtype=np.float32)
    Q = np.ascontiguousarray(np.asarray(Q), dtype=np.float32)
    w = np.asarray(w, dtype=np.float32)
    b = np.asarray(b, dtype=np.float32)
    B, Lc, d = C.shape
    Lq = Q.shape[1]
    NB = B // N_CORES
    NT, NJ = Lc // 128, Lq // 128

    nc = _get_nc(NB, Lc, Lq)

    C16 = C.astype(np.float16)
    Q16 = Q.astype(np.float16)
    CTh = np.ascontiguousarray(C16.transpose(0, 2, 1))
    QTh = np.ascontiguousarray(Q16.transpose(0, 2, 1))
    CNp = np.ascontiguousarray(
        C16.reshape(B, NT, 128, d).transpose(0, 2, 1, 3).reshape(B, 128, NT * d))
    wc = np.ascontiguousarray(w[d:2 * d].reshape(d, 1))
    wm = np.ascontiguousarray(w[2 * d:].reshape(d, 1))
    br = np.full((d, 1), b[0], dtype=np.float32)
    qb = (Q @ w[:d] + b[0]).astype(np.float32)       # (B, Lq)
    QBp = np.ascontiguousarray(
        qb.reshape(B, NJ, 128).transpose(0, 2, 1))   # (B, 128, NJ)
    QBc = np.ascontiguousarray(
        QBp.reshape(N_CORES, NB, 128, NJ).transpose(0, 2, 1, 3)
        .reshape(N_CORES, 128, NB * NJ))             # per-core packed

    in_maps = []
    for c in range(N_CORES):
        s = slice(c * NB, (c + 1) * NB)
        in_maps.append({
            "CT": CTh[s], "CN": CNp[s], "QT": QTh[s], "QB": QBc[c],
            "WC": wc, "WM": wm, "BR": br,
        })
    res = run_bass_kernel_spmd(nc, in_maps, core_ids=list(range(N_CORES)))
    global LAST_RESULTS
    LAST_RESULTS = res

    out = np.empty((B, Lc, 4 * d), dtype=np.float32)
    out[:, :, 0:d] = C
    for c in range(N_CORES):
        out[c * NB:(c + 1) * NB, :, d:] = res.results[c]["OUT"].astype(np.float32)
    return out


# revision 26
# speedup vs baseline: 1.0269x; 1.0269x over previous
"""CQAttention Trainium2 kernel (fp16 fast path).

Full inputs -> full output; internally data-parallel over batch B=32 across
8 NeuronCores (NB=4 batch items per core).

Math (per batch item, d=128, Lc=2048, Lq=256, all-ones masks):
  S[i,j] = (C@w_c)[i] + (Q@w_q)[j] + b + (C*w_m)[i] @ Q[j]
  E = exp(S); s1_j = sum_i E; s2_i = sum_j E
  C2Q = (E/s1) @ Q ; T = (E/s2)^T @ C ; Q2C = (E/s1) @ T
  out = concat([C, C2Q, C*C2Q, C*Q2C], -1)

Device decomposition (exp without max-subtraction is safe: |S| <~ 6):
  qm'[d,j] = w_m[d]*Q[j,d] + w_c[d]   (so qm'^T C^T = S_mm + r_i rides the MM)
  ht[j,i]  = exp(qm'^T@C^T + qb_j + b) = E^T   (ACT exp, bias per partition,
             accum -> s1)
  G[i,j]   = PE-transpose of ht (no second exp pass); s2 = DVE reduce of G
  T^T[d,j] = (C/s2)^T @ G ; Tw[j,d] = T/s1 via PE transpose + scale
  [C2Q|Q2C][i,:] = sum_j ht[j,i]*[Q/s1 | Tw][j,:]   (er/eq factors inside E)
  col2 = C*C2Q (gpsimd), col3 = C*Q2C (DVE, from PSUM)

All matmuls run in float16 (1 PE cycle/row at 2.4 GHz). I/O is fp16; host
converts. Tolerance is 2e-2 fro; fp16 end-to-end lands ~1e-3.
"""

import numpy as np

import concourse.bass as bass
import concourse.mybir as mybir
import concourse.tile as tile
import concourse.bacc as bacc
from concourse import masks as cmasks
from concourse.bass_utils import run_bass_kernel_spmd

F32 = mybir.dt.float32
F16 = mybir.dt.float16
AF = mybir.ActivationFunctionType
ALU = mybir.AluOpType
AX = mybir.AxisListType

N_CORES = 8
D = 128


def build_nc(NB=4, Lc=2048, Lq=256):
    NT = Lc // 128   # 16 i-tiles
    NJ = Lq // 128   # 2 j-tiles

    nc = bacc.Bacc()
    CT = nc.declare_dram_parameter("CT", [NB, 128, Lc], F16, isOutput=False)
    CN = nc.declare_dram_parameter("CN", [NB, 128, Lc], F16, isOutput=False)
    QT = nc.declare_dram_parameter("QT", [NB, 128, Lq], F16, isOutput=False)
    WC = nc.declare_dram_parameter("WC", [128, 1], F32, isOutput=False)
    WM = nc.declare_dram_parameter("WM", [128, 1], F32, isOutput=False)
    QB = nc.declare_dram_parameter("QB", [128, NB * 2], F32, isOutput=False)
    BR = nc.declare_dram_parameter("BR", [128, 1], F32, isOutput=False)
    OUT = nc.declare_dram_parameter("OUT", [NB, Lc, 384], F16, isOutput=True)

    with tile.TileContext(nc) as tc:
        import contextlib
        with contextlib.ExitStack() as ctx:
            const = ctx.enter_context(tc.tile_pool(name="const", bufs=1))
            pin = ctx.enter_context(tc.tile_pool(name="pin", bufs=4))
            mid = ctx.enter_context(tc.tile_pool(name="mid", bufs=2))
            psHT = ctx.enter_context(tc.tile_pool(name="psHT", bufs=2, space="PSUM"))
            psGT = ctx.enter_context(tc.tile_pool(name="psGT", bufs=3, space="PSUM"))
            psT = ctx.enter_context(tc.tile_pool(name="psT", bufs=1, space="PSUM"))
            psF = ctx.enter_context(tc.tile_pool(name="psF", bufs=2, space="PSUM"))

            # ---- constants ----
            wc_col = const.tile([128, 1], F32)
            nc.sync.dma_start(wc_col[:], WC[:])
            wm_col = const.tile([128, 1], F32)
            nc.sync.dma_start(wm_col[:], WM[:])
            b_rep = const.tile([128, 1], F32)
            nc.sync.dma_start(b_rep[:], BR[:])
            ones16 = const.tile([1, 128], F16)
            nc.gpsimd.memset(ones16[:], 1.0)
            wrhs = const.tile([1, 512], F16)
            nc.gpsimd.memset(wrhs[:], 1.0)
            ident = const.tile([128, 128], F16)
            cmasks.make_identity(nc, ident[:])
            qbb_all = const.tile([128, NB * 2], F32)
            nc.sync.dma_start(qbb_all[:], QB[:])

            # per-batch state handles
            st = [dict() for _ in range(NB)]

            def loads(bi):
                s = st[bi]
                qt = pin.tile([128, Lq], F16, tag="qt")
                nc.sync.dma_start(qt[:], QT[bi])
                ct = pin.tile([128, Lc], F16, tag="ct")
                nc.sync.dma_start(ct[:], CT[bi])
                cn = pin.tile([128, Lc], F16, tag="cn")
                nc.sync.dma_start(cn[:], CN[bi])
                s["qt"], s["ct"], s["cn"] = qt, ct, cn
                s["qbb"] = qbb_all[:, bi * 2:(bi + 1) * 2]

            def prep(bi):
                # qm' = qt*wm + wc ; qb cols ; qbb = qb + b
                s = st[bi]
                qm = mid.tile([128, Lq], F16, tag="qm")
                nc.gpsimd.tensor_scalar(qm[:], s["qt"][:], wm_col[:], wc_col[:],
                                        ALU.mult, ALU.add)
                ht = mid.tile([128, NJ * Lc], F16, tag="ht")
                s1p = mid.tile([128, NJ * 4], F32, tag="s1p")
                rs2 = mid.tile([128, NT], F16, tag="rs2")
                s["qm"], s["ht"], s["s1p"], s["rs2"] = qm, ht, s1p, rs2
                s["Cs"] = mid.tile([128, Lc], F16, tag="Cs", name="Cs")

            def ht_unit(bi, jj, g):
                # one 512-wide score chunk: MM + exp evac (+ s1 accum)
                s = st[bi]
                pg = psHT.tile([128, 512], F32, tag="ht")
                nc.tensor.matmul(pg[:], s["qm"][:, jj * 128:(jj + 1) * 128],
                                 s["ct"][:, g * 512:(g + 1) * 512],
                                 start=True, stop=True)
                nc.scalar.activation(
                    s["ht"][:, jj * Lc + g * 512: jj * Lc + (g + 1) * 512],
                    pg[:], AF.Exp, bias=s["qbb"][:, jj:jj + 1],
                    accum_out=s["s1p"][:, jj * 4 + g: jj * 4 + g + 1])

            def s1_fin(bi):
                s = st[bi]
                s1 = mid.tile([128, NJ], F32, tag="s1")
                nc.vector.tensor_reduce(
                    s1[:], s["s1p"][:].rearrange("p (j g) -> p j g", g=4),
                    AX.X, ALU.add)
                rs1 = mid.tile([128, NJ], F32, tag="rs1")
                nc.vector.reciprocal(rs1[:], s1[:])
                s["rs1"] = rs1

            def gt_unit(bi, c):
                # transpose 4 i-tiles (8 blocks) of ht into G via PE, then
                # per-tile DVE evac fused with s2 row-sum (TTR, 2x mode)
                s = st[bi]
                if c == 0:
                    s["G"] = mid.tile([128, NT * Lq], F16, tag="G", name="G")
                    s["s2p"] = mid.tile([128, NT], F16, tag="s2p", name="s2p")
                pg = psGT.tile([128, 1024], F16, tag="gt")
                for u in range(4):
                    t = c * 4 + u
                    for jj in range(NJ):
                        nc.tensor.matmul(
                            pg[:, u * 256 + jj * 128: u * 256 + (jj + 1) * 128],
                            s["ht"][:, jj * Lc + t * 128: jj * Lc + (t + 1) * 128],
                            ident[:], is_transpose=True)
                if c % 2 == 0:
                    nc.vector.tensor_copy(s["G"][:, c * 1024:(c + 1) * 1024],
                                          pg[:])
                else:
                    nc.scalar.activation(s["G"][:, c * 1024:(c + 1) * 1024],
                                         pg[:], AF.Copy)
                with nc.allow_low_precision("s2 ~ 4e2, fp16 out is 6e-4 rel"):
                    nc.vector.tensor_reduce(
                        s["s2p"][:, c * 4:(c + 1) * 4],
                        s["G"][:, c * 1024:(c + 1) * 1024]
                            .rearrange("p (t j) -> p t j", j=Lq),
                        AX.X, ALU.add)
                    nc.vector.reciprocal(s["rs2"][:, c * 4:(c + 1) * 4],
                                         s["s2p"][:, c * 4:(c + 1) * 4])

            def cs_unit(bi, c):
                # Cs = C / s2 for 4 i-tiles (gpsimd)
                s = st[bi]
                ts = slice(c * 4, (c + 1) * 4)
                nc.gpsimd.tensor_tensor(
                    s["Cs"][:].rearrange("p (t d) -> p t d", d=128)[:, ts, :],
                    s["cn"][:].rearrange("p (t d) -> p t d", d=128)[:, ts, :],
                    s["rs2"][:].rearrange("p t -> p t ()")[:, ts, :]
                        .broadcast_to((128, 4, 128)),
                    ALU.mult)

            def tt_unit(bi, k):
                # two T^T accumulation matmuls
                s = st[bi]
                if k == 0:
                    s["psT"] = psT.tile([128, Lq], F32, tag="tt", name="psTT")
                for t in (2 * k, 2 * k + 1):
                    nc.tensor.matmul(s["psT"][:], s["Cs"][:, t * 128:(t + 1) * 128],
                                     s["G"][:, t * Lq:(t + 1) * Lq],
                                     start=(t == 0), stop=(t == NT - 1),
                                     skip_group_check=True)

            def te_unit(bi):
                s = st[bi]
                Tt = mid.tile([128, Lq], F16, tag="Tt")
                nc.vector.tensor_copy(Tt[:], s["psT"][:])
                s["Tt"] = Tt

            def tr_unit(bi):
                # transposes: T^T -> Tw (scaled by 1/s1), qt -> Qs (scaled)
                s = st[bi]
                qtw = mid.tile([128, NJ, 256], F16, tag="qtw")
                ptrps = psGT.tile([128, 1024], F16, tag="gt", name="ptrps")
                ptr = ptrps[:, 0:256]
                for jj in range(NJ):
                    nc.tensor.matmul(ptr[:, jj * 128:(jj + 1) * 128],
                                     s["Tt"][:, jj * 128:(jj + 1) * 128],
                                     ident[:], is_transpose=True)
                pqs = ptrps[:, 512:768]
                for jj in range(NJ):
                    nc.tensor.matmul(pqs[:, jj * 128:(jj + 1) * 128],
                                     s["qt"][:, jj * 128:(jj + 1) * 128],
                                     ident[:], is_transpose=True)
                for jj in range(NJ):
                    nc.vector.tensor_scalar_mul(
                        qtw[:, jj, 128:256], ptr[:, jj * 128:(jj + 1) * 128],
                        s["rs1"][:, jj:jj + 1])
                    nc.vector.tensor_scalar_mul(
                        qtw[:, jj, 0:128], pqs[:, jj * 128:(jj + 1) * 128],
                        s["rs1"][:, jj:jj + 1])
                s["qtw"] = qtw
                s["big"] = mid.tile([128, NT, 384], F16, tag="big", name="big")

            def f_unit(bi, p):
                # fused C2Q/Q2C for tile pair (2p, 2p+1) + evac + products
                s = st[bi]
                pf = psF.tile([128, 512], F32, tag="f")
                for k in range(2):
                    t = 2 * p + k
                    for jj in range(NJ):
                        nc.tensor.matmul(
                            pf[:, k * 256:(k + 1) * 256],
                            s["ht"][:, jj * Lc + t * 128: jj * Lc + (t + 1) * 128],
                            s["qtw"][:, jj, :],
                            start=(jj == 0), stop=(jj == NJ - 1))
                pfv = pf[:].rearrange("p (k c) -> p k c", c=256)
                ts = slice(2 * p, 2 * p + 2)
                big, cn = s["big"], s["cn"]
                cnv = cn[:].rearrange("p (t d) -> p t d", d=128)
                if p >= 5:
                    nc.vector.tensor_copy(big[:, ts, 0:128], pfv[:, :, 0:128])
                else:
                    nc.scalar.activation(big[:, ts, 0:128], pfv[:, :, 0:128],
                                         AF.Copy)
                nc.vector.tensor_tensor(big[:, ts, 256:384], cnv[:, ts, :],
                                        pfv[:, :, 128:256], ALU.mult)
                nc.gpsimd.tensor_tensor(big[:, ts, 128:256], cnv[:, ts, :],
                                        big[:, ts, 0:128], ALU.mult)

            def store_unit(bi, q):
                s = st[bi]
                outv = OUT[bi].rearrange("(t p) c -> p t c", p=128)
                if bi == NB - 1 and q == 3:
                    nc.sync.dma_start(outv[:, 12:14, :], s["big"][:, 12:14, :])
                    nc.sync.dma_start(outv[:, 14:16, :], s["big"][:, 14:16, :])
                else:
                    ts = slice(q * 4, (q + 1) * 4)
                    nc.sync.dma_start(outv[:, ts, :], s["big"][:, ts, :])

            def stream1(bi):
                units = []
                for g in range(4):
                    for jj in range(NJ):
                        units.append(lambda b=bi, j=jj, g_=g: ht_unit(b, j, g_))
                    units.append(lambda b=bi, c_=g: gt_unit(b, c_))
                    if g > 0:
                        units.append(lambda b=bi, c_=g - 1: cs_unit(b, c_))
                units.append(lambda b=bi: s1_fin(b))
                units.append(lambda b=bi: cs_unit(b, 3))
                return units

            def stream2(bi):
                units = []
                for k in range(NT // 2):
                    units.append(lambda b=bi, k_=k: tt_unit(b, k_))
                units.append(lambda b=bi: te_unit(b))
                units.append(lambda b=bi: tr_unit(b))
                for p in range(NT // 2):
                    units.append(lambda b=bi, p_=p: f_unit(b, p_))
                    if p % 2 == 1:
                        units.append(lambda b=bi, q=p // 2: store_unit(b, q))
                return units

            # ---- prologue: loads for b0/b1, PE warm-up ----
            loads(0)
            for _k in range(14):
                pw = psHT.tile([128, 512], F32, tag="ht")
                nc.tensor.matmul(pw[:], ones16[:], wrhs[:], start=True, stop=True)
            loads(1)

            # ---- software-pipelined windows ----
            def interleave(a, b):
                out, ia, ib = [], 0, 0
                na, nb = len(a), len(b)
                while ia < na or ib < nb:
                    if ia < na:
                        out.append(a[ia]); ia += 1
                    if ib < nb:
                        out.append(b[ib]); ib += 1
                return out

            prep(0)
            prev = []
            for bi in range(NB):
                if bi + 2 <= NB - 1:
                    loads(bi + 2)
                if bi + 1 <= NB - 1:
                    prep(bi + 1)
                for u in interleave(stream1(bi), prev):
                    u()
                prev = stream2(bi)
            for u in prev:
                u()

    nc.finalize()
    return nc


_NC_CACHE = {}
LAST_RESULTS = None


def _get_nc(NB, Lc, Lq):
    key = (NB, Lc, Lq)
    if key not in _NC_CACHE:
        _NC_CACHE[key] = build_nc(NB, Lc, Lq)
    return _NC_CACHE[key]


def kernel(C, Q, w, b, c_mask, q_mask):
    C = np.ascontiguousarray(np.asarray(C), dtype=np.float32)
    Q = np.ascontiguousarray(np.asarray(Q), dtype=np.float32)
    w = np.asarray(w, dtype=np.float32)
    b = np.asarray(b, dtype=np.float32)
    B, Lc, d = C.shape
    Lq = Q.shape[1]
    NB = B // N_CORES
    NT, NJ = Lc // 128, Lq // 128

    nc = _get_nc(NB, Lc, Lq)

    C16 = C.astype(np.float16)
    Q16 = Q.astype(np.float16)
    CTh = np.ascontiguousarray(C16.transpose(0, 2, 1))
    QTh = np.ascontiguousarray(Q16.transpose(0, 2, 1))
    CNp = np.ascontiguousarray(
        C16.reshape(B, NT, 128, d).transpose(0, 2, 1, 3).reshape(B, 128, NT * d))
    wc = np.ascontiguousarray(w[d:2 * d].reshape(d, 1))
    wm = np.ascontiguousarray(w[2 * d:].reshape(d, 1))
    br = np.full((d, 1), b[0], dtype=np.float32)
    qb = (Q @ w[:d] + b[0]).astype(np.float32)       # (B, Lq)
    QBp = np.ascontiguousarray(
        qb.reshape(B, NJ, 128).transpose(0, 2, 1))   # (B, 128, NJ)
    QBc = np.ascontiguousarray(
        QBp.reshape(N_CORES, NB, 128, NJ).transpose(0, 2, 1, 3)
        .reshape(N_CORES, 128, NB * NJ))             # per-core packed

    in_maps = []
    for c in range(N_CORES):
        s = slice(c * NB, (c + 1) * NB)
        in_maps.append({
            "CT": CTh[s], "CN": CNp[s], "QT": QTh[s], "QB": QBc[c],
            "WC": wc, "WM": wm, "BR": br,
        })
    res = run_bass_kernel_spmd(nc, in_maps, core_ids=list(range(N_CORES)))
    global LAST_RESULTS
    LAST_RESULTS = res

    out = np.empty((B, Lc, 4 * d), dtype=np.float32)
    out[:, :, 0:d] = C
    for c in range(N_CORES):
        out[c * NB:(c + 1) * NB, :, d:] = res.results[c]["OUT"].astype(np.float32)
    return out
